# revision 5
# baseline (speedup 1.0000x reference)
"""BinaryLinear kernel for Trainium2 (8 NeuronCores, SPMD).

Computes  out = sign(x) @ sign(W)^T * alpha  for
x: [8192, 2048] f32, W: [2048, 2048] f32, alpha: [1] f32.

Strategy: data-parallel over the token dim (8 shards of 1024 tokens);
W replicated. Host side pre-transposes x-shards and W so the device
sees the contraction dim (in_features) on partitions. On device:
sign() both operands into resident bf16 SBUF buffers (+-1 is exact in
bf16, and accumulation of <=2048 +-1 terms is exact in fp32 PSUM),
then a dense PE matmul, scale by alpha, write out.

Scheduling: DMA issue order matches consumption order (x[k] and
W[k,n0] interleaved, then W[k,n1], W[k,n2], W[k,n3] chunks), spread
over three DGE rings (x on gpsimd/SWDGE, W on sync, outputs+alpha on
scalar). Matmul loop is n-outer / m / k-inner so PSUM drains spread
across each pass; the last pass's output is written per-m-pair so the
tail stays short.
"""

import numpy as np

import concourse.bass as bass
import concourse.tile as tile
from concourse import bacc, mybir
from concourse.bass_utils import run_bass_kernel_spmd

N_CORES = 8
NTOK = 8192
INF = 2048
OUTF = 2048
TPC = NTOK // N_CORES  # tokens per core (1024)
P = 128
KT = INF // P  # 16 contraction tiles
MT = TPC // P  # 8 token tiles per core
NTS = 512  # out_features per matmul (one PSUM bank)
NT = OUTF // NTS  # 4

F32 = mybir.dt.float32
BF16 = mybir.dt.bfloat16

_compiled = None
LAST_RESULT = None  # BassKernelResults of the most recent run (for profiling)


def _build():
    nc = bacc.Bacc(
        "TRN2",
        target_bir_lowering=False,
        debug=False,
        num_devices=N_CORES,
    )
    xt = nc.dram_tensor("xt", [INF, TPC], F32, kind="ExternalInput").ap()
    wt = nc.dram_tensor("wt", [INF, OUTF], F32, kind="ExternalInput").ap()
    al = nc.dram_tensor("alpha", [P, 1], F32, kind="ExternalInput").ap()
    out = nc.dram_tensor("out", [TPC, OUTF], F32, kind="ExternalOutput").ap()

    # [128, k, .] / [128, m, .] views of the DRAM tensors
    wt_r = wt.rearrange("(k p) c -> p k c", p=P)  # [128, 16, 2048]
    xt_r = xt.rearrange("(k p) c -> p k c", p=P)  # [128, 16, 1024]
    out_r = out.rearrange("(m p) c -> p m c", p=P)  # [128, 8, 2048]

    with tile.TileContext(nc) as tc:
        with (
            tc.tile_pool(name="res", bufs=1) as res,
            tc.tile_pool(name="wload", bufs=6) as wload,
            tc.tile_pool(name="xload", bufs=3) as xload,
            tc.tile_pool(name="psum", bufs=4, space="PSUM") as ppool,
            tc.tile_pool(name="outp", bufs=2) as outp,
        ):
            alpha_t = res.tile([P, 1], F32)
            nc.scalar.dma_start(alpha_t[:], al)

            # Resident sign() buffers (bf16)
            bw = res.tile([P, KT, OUTF], BF16)  # 64 KB/partition
            bx = res.tile([P, KT, TPC], BF16)  # 32 KB/partition

            def load_sign_w_chunk(k, n):
                wf = wload.tile([P, NTS], F32, name="wf", tag="wf")
                nc.sync.dma_start(wf[:], wt_r[:, k, n * NTS : (n + 1) * NTS])
                # ACT: sign(w chunk) -> bf16
                nc.scalar.sign(bw[:, k, n * NTS : (n + 1) * NTS], wf[:])

            # ---- load + sign phase (issue order == consumption order) ----
            for k in range(KT):
                xf = xload.tile([P, TPC], F32)
                nc.gpsimd.dma_start(xf[:], xt_r[:, k, :])
                # DVE: sign(x) as (x > 0) -> {1,0} bf16, then in-place *2-1
                nc.vector.tensor_scalar(
                    bx[:, k, :], xf[:], 0.0, None, op0=mybir.AluOpType.is_gt
                )
                nc.vector.tensor_scalar(
                    bx[:, k, :],
                    bx[:, k, :],
                    2.0,
                    -1.0,
                    op0=mybir.AluOpType.mult,
                    op1=mybir.AluOpType.add,
                )
                load_sign_w_chunk(k, 0)
            for n in range(1, NT):
                for k in range(KT):
                    load_sign_w_chunk(k, n)

            # ---- matmul phase: n-outer, m-middle, k-inner ----
            for n in range(NT):
                obuf = outp.tile([P, MT, NTS], F32)
                for m in range(MT):
                    ps = ppool.tile([P, NTS], F32, name="ps", tag="ps")
                    for k in range(KT):
                        nc.tensor.matmul(
                            ps[:],
                            bx[:, k, m * P : (m + 1) * P],
                            bw[:, k, n * NTS : (n + 1) * NTS],
                            start=(k == 0),
                            stop=(k == KT - 1),
                        )
                    # DVE: scale by alpha while draining PSUM -> SBUF
                    nc.vector.tensor_scalar_mul(obuf[:, m, :], ps[:], alpha_t[:])
                    if n == NT - 1 and m % 2 == 1:
                        # last pass: write per m-pair to keep the tail short
                        nc.scalar.dma_start(
                            out_r[:, m - 1 : m + 1, n * NTS : (n + 1) * NTS],
                            obuf[:, m - 1 : m + 1, :],
                        )
                if n < NT - 1:
                    nc.scalar.dma_start(
                        out_r[:, :, n * NTS : (n + 1) * NTS], obuf[:]
                    )

    nc.compile()
    return nc


def kernel(x, weight, alpha):
    global _compiled, LAST_RESULT
    if _compiled is None:
        _compiled = _build()
    nc = _compiled

    x = np.asarray(x, dtype=np.float32)
    weight = np.asarray(weight, dtype=np.float32)
    alpha = np.asarray(alpha, dtype=np.float32)

    wt = np.ascontiguousarray(weight.T)
    alv = np.full((P, 1), alpha.reshape(-1)[0], dtype=np.float32)
    in_maps = []
    for c in range(N_CORES):
        xs = np.ascontiguousarray(x[c * TPC : (c + 1) * TPC, :].T)
        in_maps.append({"xt": xs, "wt": wt, "alpha": alv})

    LAST_RESULT = run_bass_kernel_spmd(nc, in_maps, list(range(N_CORES)))
    outs = [LAST_RESULT.results[c]["out"] for c in range(N_CORES)]
    return np.concatenate(outs, axis=0)


# revision 7
# speedup vs baseline: 1.1214x; 1.1214x over previous
"""BinaryLinear kernel for Trainium2 (8 NeuronCores, SPMD).

Computes  out = sign(x) @ sign(W)^T * alpha  for
x: [8192, 2048] f32, W: [2048, 2048] f32, alpha: [1] f32.

Strategy: data-parallel over the token dim (8 shards of 1024 tokens);
W replicated. Host side pre-transposes x-shards and W so the device
sees the contraction dim (in_features) on partitions. On device:
sign() both operands into resident bf16 SBUF buffers (+-1 is exact in
bf16, and accumulation of <=2048 +-1 terms is exact in fp32 PSUM),
then a dense PE matmul, scale by alpha, write out.

Scheduling: DMA issue order matches consumption order (x[k] and
W[k,n0] interleaved, then W[k,n1], W[k,n2], W[k,n3] chunks), spread
over three DGE rings (x on gpsimd/SWDGE, W on sync, outputs+alpha on
scalar). Matmul loop is n-outer / m / k-inner so PSUM drains spread
across each pass; the last pass's output is written per-m-pair so the
tail stays short.
"""

import numpy as np

import concourse.bass as bass
import concourse.tile as tile
from concourse import bacc, mybir
from concourse.bass_utils import run_bass_kernel_spmd

N_CORES = 8
NTOK = 8192
INF = 2048
OUTF = 2048
TPC = NTOK // N_CORES  # tokens per core (1024)
P = 128
KT = INF // P  # 16 contraction tiles
MT = TPC // P  # 8 token tiles per core
NTS = 512  # out_features per matmul (one PSUM bank)
NT = OUTF // NTS  # 4

F32 = mybir.dt.float32
BF16 = mybir.dt.bfloat16

_compiled = None
LAST_RESULT = None  # BassKernelResults of the most recent run (for profiling)


def _build():
    nc = bacc.Bacc(
        "TRN2",
        target_bir_lowering=False,
        debug=False,
        num_devices=N_CORES,
    )
    xt = nc.dram_tensor("xt", [INF, TPC], F32, kind="ExternalInput").ap()
    wt = nc.dram_tensor("wt", [INF, OUTF], F32, kind="ExternalInput").ap()
    al = nc.dram_tensor("alpha", [P, 1], F32, kind="ExternalInput").ap()
    out = nc.dram_tensor("out", [TPC, OUTF], F32, kind="ExternalOutput").ap()

    # [128, k, .] / [128, m, .] views of the DRAM tensors
    wt_r = wt.rearrange("(k p) c -> p k c", p=P)  # [128, 16, 2048]
    xt_r = xt.rearrange("(k p) c -> p k c", p=P)  # [128, 16, 1024]
    out_r = out.rearrange("(m p) c -> p m c", p=P)  # [128, 8, 2048]

    with tile.TileContext(nc) as tc:
        with (
            tc.tile_pool(name="res", bufs=1) as res,
            tc.tile_pool(name="wload", bufs=6) as wload,
            tc.tile_pool(name="xload", bufs=3) as xload,
            tc.tile_pool(name="psum", bufs=8, space="PSUM") as ppool,
            tc.tile_pool(name="outp", bufs=2) as outp,
        ):
            alpha_t = res.tile([P, 1], F32)
            nc.scalar.dma_start(alpha_t[:], al)

            # Resident sign() buffers (bf16)
            bw = res.tile([P, KT, OUTF], BF16)  # 64 KB/partition
            bx = res.tile([P, KT, TPC], BF16)  # 32 KB/partition

            def load_sign_w_chunk(k, n):
                wf = wload.tile([P, NTS], F32, name="wf", tag="wf")
                nc.sync.dma_start(wf[:], wt_r[:, k, n * NTS : (n + 1) * NTS])
                # ACT: sign(w chunk) -> bf16
                nc.scalar.sign(bw[:, k, n * NTS : (n + 1) * NTS], wf[:])

            # ---- load + sign phase (issue order == consumption order) ----
            for k in range(KT):
                xf = xload.tile([P, TPC], F32)
                nc.gpsimd.dma_start(xf[:], xt_r[:, k, :])
                # DVE: sign(x) as (x > 0) -> {1,0} bf16, then in-place *2-1
                nc.vector.tensor_scalar(
                    bx[:, k, :], xf[:], 0.0, None, op0=mybir.AluOpType.is_gt
                )
                nc.vector.tensor_scalar(
                    bx[:, k, :],
                    bx[:, k, :],
                    2.0,
                    -1.0,
                    op0=mybir.AluOpType.mult,
                    op1=mybir.AluOpType.add,
                )
                load_sign_w_chunk(k, 0)
            for n in range(1, NT):
                for k in range(KT):
                    load_sign_w_chunk(k, n)

            # ---- matmul phase ----
            # Early passes (n0, n1) run while inputs stream in: k-middle /
            # m-inner so each arriving k-tile feeds 8 matmuls at once.
            # Resident passes (n2, n3) run m-outer / k-inner so PSUM drains
            # and output writes spread across the pass (short tail).
            for n in range(NT):
                obuf = outp.tile([P, MT, NTS], F32)
                if n < 2:
                    pss = [
                        ppool.tile([P, NTS], F32, name="ps", tag="ps")
                        for _ in range(MT)
                    ]
                    for k in range(KT):
                        for m in range(MT):
                            nc.tensor.matmul(
                                pss[m][:],
                                bx[:, k, m * P : (m + 1) * P],
                                bw[:, k, n * NTS : (n + 1) * NTS],
                                start=(k == 0),
                                stop=(k == KT - 1),
                            )
                    for m in range(MT):
                        nc.vector.tensor_scalar_mul(
                            obuf[:, m, :], pss[m][:], alpha_t[:]
                        )
                    nc.scalar.dma_start(
                        out_r[:, :, n * NTS : (n + 1) * NTS], obuf[:]
                    )
                else:
                    for m in range(MT):
                        ps = ppool.tile([P, NTS], F32, name="ps", tag="ps")
                        for k in range(KT):
                            nc.tensor.matmul(
                                ps[:],
                                bx[:, k, m * P : (m + 1) * P],
                                bw[:, k, n * NTS : (n + 1) * NTS],
                                start=(k == 0),
                                stop=(k == KT - 1),
                            )
                        nc.vector.tensor_scalar_mul(obuf[:, m, :], ps[:], alpha_t[:])
                        if m % 2 == 1:
                            nc.scalar.dma_start(
                                out_r[:, m - 1 : m + 1, n * NTS : (n + 1) * NTS],
                                obuf[:, m - 1 : m + 1, :],
                            )

    nc.compile()
    return nc


def kernel(x, weight, alpha):
    global _compiled, LAST_RESULT
    if _compiled is None:
        _compiled = _build()
    nc = _compiled

    x = np.asarray(x, dtype=np.float32)
    weight = np.asarray(weight, dtype=np.float32)
    alpha = np.asarray(alpha, dtype=np.float32)

    wt = np.ascontiguousarray(weight.T)
    alv = np.full((P, 1), alpha.reshape(-1)[0], dtype=np.float32)
    in_maps = []
    for c in range(N_CORES):
        xs = np.ascontiguousarray(x[c * TPC : (c + 1) * TPC, :].T)
        in_maps.append({"xt": xs, "wt": wt, "alpha": alv})

    LAST_RESULT = run_bass_kernel_spmd(nc, in_maps, list(range(N_CORES)))
    outs = [LAST_RESULT.results[c]["out"] for c in range(N_CORES)]
    return np.concatenate(outs, axis=0)


# revision 11
# speedup vs baseline: 1.4057x; 1.2535x over previous
"""BinaryLinear kernel for Trainium2 (8 NeuronCores, SPMD).

Computes  out = sign(x) @ sign(W)^T * alpha  for
x: [8192, 2048] f32, W: [2048, 2048] f32, alpha: [1] f32.

Strategy: data-parallel over the token dim (8 shards of 1024 tokens);
W replicated. Host side pre-transposes x-shards and W so the device
sees the contraction dim (in_features) on partitions. On device:
sign() both operands into resident bf16 SBUF buffers (+-1 is exact in
bf16, and accumulation of <=2048 +-1 terms is exact in fp32 PSUM),
then a dense PE matmul, scale by alpha, write out.

Scheduling: DMA issue order matches consumption order (x[k] and
W[k,n0] interleaved, then W[k,n1], W[k,n2], W[k,n3] chunks), spread
over three DGE rings (x on gpsimd/SWDGE, W on sync, outputs+alpha on
scalar). Matmul loop is n-outer / m / k-inner so PSUM drains spread
across each pass; the last pass's output is written per-m-pair so the
tail stays short.
"""

import numpy as np

import concourse.bass as bass
import concourse.tile as tile
from concourse import bacc, mybir
from concourse.bass_utils import run_bass_kernel_spmd

N_CORES = 8
NTOK = 8192
INF = 2048
OUTF = 2048
TPC = NTOK // N_CORES  # tokens per core (1024)
P = 128
KT = INF // P  # 16 contraction tiles
MT = TPC // P  # 8 token tiles per core
NTS = 512  # out_features per matmul (one PSUM bank)
NT = OUTF // NTS  # 4

F32 = mybir.dt.float32
BF16 = mybir.dt.bfloat16
FP8 = mybir.dt.float8e4  # E4M3; +-1.0 is exact
SIGN_DT = FP8
K_STEP = 2  # contraction tiles per matmul (2 = fp8 DoubleRow)

_compiled = None
LAST_RESULT = None  # BassKernelResults of the most recent run (for profiling)


def _build():
    nc = bacc.Bacc(
        "TRN2",
        target_bir_lowering=False,
        debug=False,
        num_devices=N_CORES,
    )
    xt = nc.dram_tensor("xt", [INF, TPC], F32, kind="ExternalInput").ap()
    wt = nc.dram_tensor("wt", [INF, OUTF], F32, kind="ExternalInput").ap()
    al = nc.dram_tensor("alpha", [P, 1], F32, kind="ExternalInput").ap()
    out = nc.dram_tensor("out", [TPC, OUTF], F32, kind="ExternalOutput").ap()

    # [128, k, .] / [128, m, .] views of the DRAM tensors
    wt_r = wt.rearrange("(k p) c -> p k c", p=P)  # [128, 16, 2048]
    xt_r = xt.rearrange("(k p) c -> p k c", p=P)  # [128, 16, 1024]
    out_r = out.rearrange("(m p) c -> p m c", p=P)  # [128, 8, 2048]

    with tile.TileContext(nc) as tc:
        with (
            tc.tile_pool(name="res", bufs=1) as res,
            tc.tile_pool(name="wload", bufs=6) as wload,
            tc.tile_pool(name="xload", bufs=3) as xload,
            tc.tile_pool(name="psum", bufs=8, space="PSUM") as ppool,
            tc.tile_pool(name="outp", bufs=2) as outp,
        ):
            alpha_t = res.tile([P, 1], F32)
            nc.scalar.dma_start(alpha_t[:], al)

            # Resident sign() buffers (fp8/bf16)
            bw = res.tile([P, KT, OUTF], SIGN_DT)
            bx = res.tile([P, KT, TPC], SIGN_DT)

            perf_mode = (
                mybir.MatmulPerfMode.DoubleRow if K_STEP == 2 else None
            )

            def mm(ps_ap, m, n, k):
                nc.tensor.matmul(
                    ps_ap,
                    bx[:, k : k + K_STEP, m * P : (m + 1) * P],
                    bw[:, k : k + K_STEP, n * NTS : (n + 1) * NTS],
                    start=(k == 0),
                    stop=(k + K_STEP >= KT),
                    perf_mode=perf_mode,
                )

            def load_sign_w_chunk(k, n):
                wf = wload.tile([P, NTS], F32, name="wf", tag="wf")
                nc.sync.dma_start(wf[:], wt_r[:, k, n * NTS : (n + 1) * NTS])
                # ACT: sign(w chunk) -> bf16
                nc.scalar.sign(bw[:, k, n * NTS : (n + 1) * NTS], wf[:])

            # ---- load + sign phase (issue order == consumption order) ----
            for k in range(KT):
                xf = xload.tile([P, TPC], F32)
                nc.gpsimd.dma_start(xf[:], xt_r[:, k, :])
                # DVE: sign(x) as (x > 0) -> {1,0} bf16, then in-place *2-1
                nc.vector.tensor_scalar(
                    bx[:, k, :], xf[:], 0.0, None, op0=mybir.AluOpType.is_gt
                )
                nc.vector.tensor_scalar(
                    bx[:, k, :],
                    bx[:, k, :],
                    2.0,
                    -1.0,
                    op0=mybir.AluOpType.mult,
                    op1=mybir.AluOpType.add,
                )
                load_sign_w_chunk(k, 0)
            for n in range(1, NT):
                for k in range(KT):
                    load_sign_w_chunk(k, n)

            # ---- matmul phase ----
            # Early passes (n0, n1) run while inputs stream in: k-middle /
            # m-inner so each arriving k-tile feeds 8 matmuls at once.
            # Resident passes (n2, n3) run m-outer / k-inner so PSUM drains
            # and output writes spread across the pass (short tail).
            for n in range(NT):
                obuf = outp.tile([P, MT, NTS], F32)
                if n < 2:
                    pss = [
                        ppool.tile([P, NTS], F32, name="ps", tag="ps")
                        for _ in range(MT)
                    ]
                    for k in range(0, KT, K_STEP):
                        for m in range(MT):
                            mm(pss[m][:], m, n, k)
                    for m in range(MT):
                        nc.vector.tensor_scalar_mul(
                            obuf[:, m, :], pss[m][:], alpha_t[:]
                        )
                    nc.scalar.dma_start(
                        out_r[:, :, n * NTS : (n + 1) * NTS], obuf[:]
                    )
                else:
                    for m in range(MT):
                        ps = ppool.tile([P, NTS], F32, name="ps", tag="ps")
                        for k in range(0, KT, K_STEP):
                            mm(ps[:], m, n, k)
                        nc.vector.tensor_scalar_mul(obuf[:, m, :], ps[:], alpha_t[:])
                        if m % 2 == 1:
                            nc.scalar.dma_start(
                                out_r[:, m - 1 : m + 1, n * NTS : (n + 1) * NTS],
                                obuf[:, m - 1 : m + 1, :],
                            )

    nc.compile()
    return nc


def kernel(x, weight, alpha):
    global _compiled, LAST_RESULT
    if _compiled is None:
        _compiled = _build()
    nc = _compiled

    x = np.asarray(x, dtype=np.float32)
    weight = np.asarray(weight, dtype=np.float32)
    alpha = np.asarray(alpha, dtype=np.float32)

    wt = np.ascontiguousarray(weight.T)
    alv = np.full((P, 1), alpha.reshape(-1)[0], dtype=np.float32)
    in_maps = []
    for c in range(N_CORES):
        xs = np.ascontiguousarray(x[c * TPC : (c + 1) * TPC, :].T)
        in_maps.append({"xt": xs, "wt": wt, "alpha": alv})

    LAST_RESULT = run_bass_kernel_spmd(nc, in_maps, list(range(N_CORES)))
    outs = [LAST_RESULT.results[c]["out"] for c in range(N_CORES)]
    return np.concatenate(outs, axis=0)
